# revision 2
# baseline (speedup 1.0000x reference)
"""GATv2Layer (nn_GATv2Layer_42356967473536) — Trainium2 Bass kernel.

Math
----
The reference computes
    hp   = einsum('bnf,hfd->bhnd', h, W)          # per-head projections
    e    = leaky_relu(hp @ hp^T)
    attn = softmax(e, axis=-1)
    out  = hp * sum(attn, axis=-1, keepdims=True) # row-sums of softmax == 1
    out  = concat_heads(out)                      # (B, N, H*D)
    res  = alpha * out + (1 - alpha) * h

sum(softmax(x), -1) is identically 1, so the whole attention block is a
no-op and, with F == H*D == 256, the layer collapses to one matmul per
batch element:
    res_b = h_b @ M,   M = alpha * Wc + (1 - alpha) * I_256,
    Wc[f, hd] = W[hd // 64, f, hd % 64]

Precision: everything is carried in bf16 (inputs cast on host, PE
accumulates bf16 x bf16 into fp32 PSUM, PSUM->SBUF copy downcasts to
bf16, host upcasts the gathered result).  Measured rel err vs the fp32
reference ~3e-3, comfortably under the 2e-2 gate, and it halves DMA
bytes and quarters PE passes vs fp32 LOW_HIGH.

Sharding
--------
Data-parallel over batch B=8 -> one batch element per NeuronCore
(8 cores). Per core: outT_b = M^T @ h_b^T as (128f x 128d) @ (128f x
Nn) PE matmuls accumulating over the two 128-row halves of F. The host
passes [M | h_b^T] concatenated in bf16 (the contraction dim must sit
on SBUF partitions), and transposes/upcasts the (256, 2048) per-core
result on gather.

Kernel structure (raw bass Block, hand-rolled semaphores)
---------------------------------------------------------
- loads:  5 column-spans (M, then 4 node chunks) x 2 F-halves on the
  two HWDGE rings (sync + scalar) so matmuls start as soon as M + the
  first chunk land.
- PE:     a few zero-matmul warmups start the HAM busy window early;
  then 8 accumulation groups (4 node chunks x 2 d-halves), one PSUM
  bank each (no bank recycling).
- DVE:    PSUM -> SBUF bf16 downcast copies per group.
- stores: per node chunk (both d-halves) on alternating rings as soon
  as its two copies land, overlapping the store stream with the tail
  of the load stream and the remaining matmuls.
"""

import os
import sys
import types
from contextlib import ExitStack

import numpy as np
from ml_dtypes import bfloat16

B, N, F = 8, 2048, 256
H, D = 4, 64
P = 128
KO = 2                 # contraction subtiles (F = 2 * 128)
NCORES = 8
W_ALL = F + N          # hm input: [M | hT] = 2304 columns
NWARM = 3
CW = 512               # node-chunk width
NCHUNK = N // CW       # 4 chunks x 2 d-halves = 8 PSUM groups

# load column-spans of hm: M first, then one span per node chunk
SPANS = [(0, F)] + [(F + c * CW, F + (c + 1) * CW) for c in range(NCHUNK)]

_NC = None
LAST_EXEC_TIME_NS = None
LAST_TRACE_PATH = None


def _ensure_axon_ntff_hook():
    """Make run_bass_kernel_spmd(trace=True) work under axon in this image
    (antenv.axon_hooks is absent; trn_boot carries the ctypes impl)."""
    try:
        import antenv.axon_hooks  # noqa: F401
        return
    except ImportError:
        pass
    try:
        from trn_agent_boot.trn_boot import _ntff_profile_via_ctypes

        hook = _ntff_profile_via_ctypes("/opt/axon/libaxon_pjrt.so")
        mod = types.ModuleType("antenv.axon_hooks")
        mod.get_axon_ntff_profile_hook = lambda: hook
        mod.set_axon_ntff_profile_hook = lambda h: None
        sys.modules["antenv.axon_hooks"] = mod
        import concourse.bass_utils as bass_utils

        bass_utils.upload_artifacts = lambda tmpdir: tmpdir  # no S3 here
    except Exception:
        pass


def _build_nc():
    from concourse import bacc, mybir

    bf16 = mybir.dt.bfloat16
    f32 = mybir.dt.float32

    nc = bacc.Bacc()
    hm = nc.declare_dram_parameter("hm", [F, W_ALL], bf16, isOutput=False)
    outT = nc.declare_dram_parameter("outT", [F, N], bf16, isOutput=True)

    hm_r = hm.rearrange("(ko p) n -> p ko n", p=P)     # (128, 2, 2304)
    oT_r = outT.rearrange("(dh p) n -> p dh n", p=P)   # (128, 2, 2048)

    with ExitStack() as es:
        h_sb = es.enter_context(nc.sbuf_tensor("h_sb", [P, KO, W_ALL], bf16))
        o_sb = es.enter_context(nc.sbuf_tensor("o_sb", [P, KO, N], bf16))
        wu_sb = es.enter_context(nc.sbuf_tensor("wu_sb", [P, 512], bf16))
        psum = [
            es.enter_context(nc.psum_tensor(f"psum{i}", [P, CW], f32))
            for i in range(8)
        ]
        sp_sems = [
            es.enter_context(nc.semaphore(f"sp_sem{s}")) for s in range(len(SPANS))
        ]
        wu_sem = es.enter_context(nc.semaphore("wu_sem"))
        mm_sem = es.enter_context(nc.semaphore("mm_sem"))
        cp_sem = es.enter_context(nc.semaphore("cp_sem"))
        st_sem = es.enter_context(nc.semaphore("st_sem"))
        blk = es.enter_context(nc.Block())

        @blk.sync
        def _(sync):
            for si, (a, b) in enumerate(SPANS):  # ko=0 halves
                sync.dma_start(h_sb[:, 0, a:b], hm_r[:, 0, a:b]).then_inc(
                    sp_sems[si], 16
                )
            # chunks 0 and 2 on this ring
            for c in (0, 2):
                sync.wait_ge(cp_sem, 2 * c + 2)
                sync.dma_start(
                    oT_r[:, :, c * CW:(c + 1) * CW],
                    o_sb[:, :, c * CW:(c + 1) * CW],
                ).then_inc(st_sem, 16)
            sync.wait_ge(st_sem, 64)  # all stores landed before kernel exit

        @blk.scalar
        def _(scalar):
            for si, (a, b) in enumerate(SPANS):  # ko=1 halves
                scalar.dma_start(h_sb[:, 1, a:b], hm_r[:, 1, a:b]).then_inc(
                    sp_sems[si], 16
                )
            # chunks 1 and 3 on this ring
            for c in (1, 3):
                scalar.wait_ge(cp_sem, 2 * c + 2)
                scalar.dma_start(
                    oT_r[:, :, c * CW:(c + 1) * CW],
                    o_sb[:, :, c * CW:(c + 1) * CW],
                ).then_inc(st_sem, 16)

        @blk.vector
        def _(vector):
            nc.vector.memset(wu_sb[:], 0.0).then_inc(wu_sem, 1)
            g = 0
            for c in range(NCHUNK):
                for dh in range(KO):
                    nc.vector.tensor_copy(
                        o_sb[:, dh, c * CW:(c + 1) * CW], psum[g][:, :]
                    )._wait_ge(mm_sem, g + 1).then_inc(cp_sem, 1)
                    g += 1

        @blk.tensor
        def _(tensor):
            tensor.wait_ge(wu_sem, 1)
            for _ in range(NWARM):  # start the HAM busy window early
                nc.tensor.matmul(
                    psum[0][:], wu_sb[:, :P], wu_sb[:], start=True, stop=True
                )
            tensor.wait_ge(sp_sems[0], 32)  # M (both ko halves)
            g = 0
            for c in range(NCHUNK):
                tensor.wait_ge(sp_sems[c + 1], 32)  # node chunk, both halves
                col = F + c * CW
                for dh in range(KO):
                    nc.tensor.matmul(
                        psum[g][:, :],
                        h_sb[:, 0, dh * P:(dh + 1) * P],
                        h_sb[:, 0, col:col + CW],
                        start=True,
                        stop=False,
                    )
                    nc.tensor.matmul(
                        psum[g][:, :],
                        h_sb[:, 1, dh * P:(dh + 1) * P],
                        h_sb[:, 1, col:col + CW],
                        start=False,
                        stop=True,
                    ).then_inc(mm_sem, 1)
                    g += 1

    nc.finalize()
    return nc


def kernel(h, adj, W, alpha_res):
    global _NC, LAST_EXEC_TIME_NS, LAST_TRACE_PATH

    h = np.asarray(h, dtype=np.float32)
    W = np.asarray(W, dtype=np.float32)
    alpha = float(np.asarray(alpha_res))
    # adj is unused by the reference's math.

    # M = alpha * concat-heads(W) + (1 - alpha) * I  (residual folded in)
    Wc = W.transpose(1, 0, 2).reshape(F, F)
    Mmat = (alpha * Wc + (1.0 - alpha) * np.eye(F, dtype=np.float32)).astype(
        bfloat16
    )

    trace = os.environ.get("BASS_TRACE", "").lower() in ("1", "true", "yes")
    if trace:
        _ensure_axon_ntff_hook()

    from concourse.bass_utils import run_bass_kernel_spmd

    if _NC is None:
        _NC = _build_nc()

    in_maps = [
        {"hm": np.concatenate([Mmat, h[b].T.astype(bfloat16)], axis=1)}
        for b in range(NCORES)
    ]
    res = run_bass_kernel_spmd(
        _NC, in_maps, core_ids=list(range(NCORES)), trace=trace
    )
    LAST_EXEC_TIME_NS = res.exec_time_ns
    if res.instructions_and_trace is not None:
        LAST_TRACE_PATH = res.instructions_and_trace[1]

    return np.ascontiguousarray(
        np.stack(
            [res.results[b]["outT"].T.astype(np.float32) for b in range(NCORES)]
        )
    )


# revision 3
# speedup vs baseline: 1.0170x; 1.0170x over previous
"""GATv2Layer (nn_GATv2Layer_42356967473536) — Trainium2 Bass kernel.

Math
----
The reference computes
    hp   = einsum('bnf,hfd->bhnd', h, W)          # per-head projections
    e    = leaky_relu(hp @ hp^T)
    attn = softmax(e, axis=-1)
    out  = hp * sum(attn, axis=-1, keepdims=True) # row-sums of softmax == 1
    out  = concat_heads(out)                      # (B, N, H*D)
    res  = alpha * out + (1 - alpha) * h

sum(softmax(x), -1) is identically 1, so the whole attention block is a
no-op and, with F == H*D == 256, the layer collapses to one matmul per
batch element:
    res_b = h_b @ M,   M = alpha * Wc + (1 - alpha) * I_256,
    Wc[f, hd] = W[hd // 64, f, hd % 64]

Precision: everything is carried in bf16 (inputs cast on host, PE
accumulates bf16 x bf16 into fp32 PSUM, PSUM->SBUF copy downcasts to
bf16, host upcasts the gathered result).  Measured rel err vs the fp32
reference ~3e-3, comfortably under the 2e-2 gate, and it halves DMA
bytes and quarters PE passes vs fp32 LOW_HIGH.

Sharding
--------
Data-parallel over batch B=8 -> one batch element per NeuronCore.
Per core: outT_b = M^T @ h_b^T as (128f x 128d) @ (128f x Nn) PE
matmuls accumulating over the two 128-row halves of F.

Layout / schedule
-----------------
The host packs one DRAM tensor hm[128, 4608] whose per-partition
columns are, in consumption order:
    [M_ko0(256) | M_ko1(256) | c0_ko0(512) | c0_ko1(512) | c1... c3]
so every load is a single contiguous run per partition, and loads can
be issued in exactly the order the PE consumes them.  Loads all go on
the sync (SP) HWDGE ring: L0 = M + chunk0, L1 = chunk1, L2 = chunks
2+3 — FIFO on one ring means chunk0 finishes first instead of
round-robining with later data.  Stores go out per chunk as soon as
its two PSUM->SBUF copies land, on the scalar (ACT) ring (chunks
0,1,2) and the idle sync ring (chunk 3) so the two final store
receipts overlap.  PE warms the HAM clock gate with zero-matmuls
sized to end right as the first chunk lands.  Block(no_gpsimd_drain)
skips the GpSimd dge_drain in the block-end barrier (no SWDGE DMAs
are used).
"""

import os
import sys
import types
from contextlib import ExitStack

import numpy as np
from ml_dtypes import bfloat16

B, N, F = 8, 2048, 256
H, D = 4, 64
P = 128
KO = 2                 # contraction subtiles (F = 2 * 128)
NCORES = 8
NWARM = 8
CW = 512               # node-chunk width
NCHUNK = N // CW
WCOLS = KO * F + KO * N    # 4608 packed columns per partition

# packed-column helpers
def mcol(ko, dh):
    return ko * F + dh * P

def ccol(c, ko):
    return KO * F + c * KO * CW + ko * CW

# loads: (start_col, end_col, sem index). PE chunk c waits load LD_OF[c].
LOADS = [(0, ccol(1, 0)), (ccol(1, 0), ccol(2, 0)), (ccol(2, 0), WCOLS)]
LD_OF = [0, 1, 2, 2]

_NC = None
LAST_EXEC_TIME_NS = None
LAST_TRACE_PATH = None


def _ensure_axon_ntff_hook():
    """Make run_bass_kernel_spmd(trace=True) work under axon in this image
    (antenv.axon_hooks is absent; trn_boot carries the ctypes impl)."""
    try:
        import antenv.axon_hooks  # noqa: F401
        return
    except ImportError:
        pass
    try:
        from trn_agent_boot.trn_boot import _ntff_profile_via_ctypes

        hook = _ntff_profile_via_ctypes("/opt/axon/libaxon_pjrt.so")
        mod = types.ModuleType("antenv.axon_hooks")
        mod.get_axon_ntff_profile_hook = lambda: hook
        mod.set_axon_ntff_profile_hook = lambda h: None
        sys.modules["antenv.axon_hooks"] = mod
        import concourse.bass_utils as bass_utils

        bass_utils.upload_artifacts = lambda tmpdir: tmpdir  # no S3 here
    except Exception:
        pass


def _build_nc():
    from concourse import bacc, mybir

    bf16 = mybir.dt.bfloat16
    f32 = mybir.dt.float32

    nc = bacc.Bacc()
    hm = nc.declare_dram_parameter("hm", [P, WCOLS], bf16, isOutput=False)
    outT = nc.declare_dram_parameter("outT", [F, N], bf16, isOutput=True)

    oT_r = outT.rearrange("(dh p) n -> p dh n", p=P)   # (128, 2, 2048)

    with ExitStack() as es:
        h_sb = es.enter_context(nc.sbuf_tensor("h_sb", [P, WCOLS], bf16))
        o_sb = es.enter_context(nc.sbuf_tensor("o_sb", [P, KO, N], bf16))
        wu_sb = es.enter_context(nc.sbuf_tensor("wu_sb", [P, 512], bf16))
        psum = [
            es.enter_context(nc.psum_tensor(f"psum{i}", [P, CW], f32))
            for i in range(8)
        ]
        ld_sems = [
            es.enter_context(nc.semaphore(f"ld_sem{s}")) for s in range(len(LOADS))
        ]
        wu_sem = es.enter_context(nc.semaphore("wu_sem"))
        mm_sem = es.enter_context(nc.semaphore("mm_sem"))
        cp_sem = es.enter_context(nc.semaphore("cp_sem"))
        st_sem = es.enter_context(nc.semaphore("st_sem"))
        blk = es.enter_context(nc.Block(no_gpsimd_drain=True))

        @blk.sync
        def _(sync):
            for si, (a, b) in enumerate(LOADS):
                sync.dma_start(h_sb[:, a:b], hm[:, a:b]).then_inc(ld_sems[si], 16)
            # chunk 3's store on this (otherwise idle) ring so its receipt
            # overlaps chunk 2's on the scalar ring
            sync.wait_ge(cp_sem, 8)
            sync.dma_start(
                oT_r[:, :, 3 * CW:4 * CW], o_sb[:, :, 3 * CW:4 * CW]
            ).then_inc(st_sem, 16)
            sync.wait_ge(st_sem, 64)  # all stores landed before kernel exit

        @blk.scalar
        def _(scalar):
            for c in range(3):
                scalar.wait_ge(cp_sem, 2 * c + 2)
                scalar.dma_start(
                    oT_r[:, :, c * CW:(c + 1) * CW],
                    o_sb[:, :, c * CW:(c + 1) * CW],
                ).then_inc(st_sem, 16)

        @blk.vector
        def _(vector):
            nc.vector.memset(wu_sb[:], 0.0).then_inc(wu_sem, 1)
            g = 0
            for c in range(NCHUNK):
                for dh in range(KO):
                    nc.vector.tensor_copy(
                        o_sb[:, dh, c * CW:(c + 1) * CW], psum[g][:, :]
                    )._wait_ge(mm_sem, g + 1).then_inc(cp_sem, 1)
                    g += 1

        @blk.tensor
        def _(tensor):
            tensor.wait_ge(wu_sem, 1)
            for _ in range(NWARM):  # HAM warm-up on zeros while loads fly
                nc.tensor.matmul(
                    psum[0][:], wu_sb[:, :P], wu_sb[:], start=True, stop=True
                )
            g = 0
            for c in range(NCHUNK):
                tensor.wait_ge(ld_sems[LD_OF[c]], 16)
                for dh in range(KO):
                    nc.tensor.matmul(
                        psum[g][:, :],
                        h_sb[:, mcol(0, dh):mcol(0, dh) + P],
                        h_sb[:, ccol(c, 0):ccol(c, 0) + CW],
                        start=True,
                        stop=False,
                    )
                    nc.tensor.matmul(
                        psum[g][:, :],
                        h_sb[:, mcol(1, dh):mcol(1, dh) + P],
                        h_sb[:, ccol(c, 1):ccol(c, 1) + CW],
                        start=False,
                        stop=True,
                    ).then_inc(mm_sem, 1)
                    g += 1

    nc.finalize()
    return nc


def _pack_inputs(h, Mmat_bf):
    """Per-core hm[128, 4608]: [M_ko0|M_ko1| c0_ko0|c0_ko1| ... c3_ko1]."""
    m_part = Mmat_bf.reshape(KO, P, F).transpose(1, 0, 2).reshape(P, KO * F)
    maps = []
    for b in range(NCORES):
        ht = np.ascontiguousarray(h[b].T).astype(bfloat16)       # (256, 2048)
        cpart = (
            ht.reshape(KO, P, NCHUNK, CW)
            .transpose(1, 2, 0, 3)
            .reshape(P, KO * N)
        )
        maps.append({"hm": np.concatenate([m_part, cpart], axis=1)})
    return maps


def kernel(h, adj, W, alpha_res):
    global _NC, LAST_EXEC_TIME_NS, LAST_TRACE_PATH

    h = np.asarray(h, dtype=np.float32)
    W = np.asarray(W, dtype=np.float32)
    alpha = float(np.asarray(alpha_res))
    # adj is unused by the reference's math.

    # M = alpha * concat-heads(W) + (1 - alpha) * I  (residual folded in)
    Wc = W.transpose(1, 0, 2).reshape(F, F)
    Mmat_bf = (alpha * Wc + (1.0 - alpha) * np.eye(F, dtype=np.float32)).astype(
        bfloat16
    )

    trace = os.environ.get("BASS_TRACE", "").lower() in ("1", "true", "yes")
    if trace:
        _ensure_axon_ntff_hook()

    from concourse.bass_utils import run_bass_kernel_spmd

    if _NC is None:
        _NC = _build_nc()

    in_maps = _pack_inputs(h, Mmat_bf)
    res = run_bass_kernel_spmd(
        _NC, in_maps, core_ids=list(range(NCORES)), trace=trace
    )
    LAST_EXEC_TIME_NS = res.exec_time_ns
    if res.instructions_and_trace is not None:
        LAST_TRACE_PATH = res.instructions_and_trace[1]

    return np.ascontiguousarray(
        np.stack(
            [res.results[b]["outT"].T.astype(np.float32) for b in range(NCORES)]
        )
    )


# revision 7
# speedup vs baseline: 1.1909x; 1.1710x over previous
"""GATv2Layer (nn_GATv2Layer_42356967473536) — Trainium2 Bass kernel.

Math
----
The reference computes
    hp   = einsum('bnf,hfd->bhnd', h, W)          # per-head projections
    e    = leaky_relu(hp @ hp^T)
    attn = softmax(e, axis=-1)
    out  = hp * sum(attn, axis=-1, keepdims=True) # row-sums of softmax == 1
    out  = concat_heads(out)                      # (B, N, H*D)
    res  = alpha * out + (1 - alpha) * h

sum(softmax(x), -1) is identically 1, so the whole attention block is a
no-op and, with F == H*D == 256, the layer collapses to one matmul per
batch element:
    res_b = h_b @ M,   M = alpha * Wc + (1 - alpha) * I_256,
    Wc[f, hd] = W[hd // 64, f, hd % 64]

Precision: bf16 end to end (PE accumulates into fp32 PSUM); measured
rel err vs the fp32 reference ~3e-3 against a 2e-2 gate.  Halves DMA
bytes and quarters PE passes vs fp32 LOW_HIGH.

Sharding
--------
Data-parallel over batch B=8 -> one batch element per NeuronCore.
Per core: outT_b = M^T @ h_b^T as (128f x 128d) @ (128f x Nn) PE
matmuls accumulating over the two 128-row halves of F.

Schedule
--------
The NEFF epilogue (NRT semaphore teardown, ~6.7us: every engine
serially clears S[3..53] after an all-engine barrier) starts only when
the LAST engine's instruction stream ends, so the whole kernel is
written to minimize max-engine-finish:
- no bass Block (its end-of-block barrier would serialize ahead of the
  NRT barrier that does the same job); instructions are emitted
  straight into main.
- no store-completion wait: the NRT epilogue drains the HWDGE queues
  and its ~6.7us of teardown runs long after the ~1us of store wire
  time, so the data is landed well before the NEFF retires.
- host packs hm[128, 4608] per-partition-contiguous in consumption
  order [M_ko0|M_ko1|c0_ko0|c0_ko1|...], loads are 3 DMAs on the sync
  ring in that order; stores go out per chunk as soon as its two
  PSUM->SBUF copies land (c0,c1 on the scalar ring, c2,c3 on sync).
- copies split between DVE and ACT so the last chunk's two copies run
  concurrently; PE warms the HAM clock gate on garbage zeros-free
  matmuls while the first load flies.
"""

import os
import sys
import types
from contextlib import ExitStack

import numpy as np
from ml_dtypes import bfloat16

B, N, F = 8, 2048, 256
H, D = 4, 64
P = 128
KO = 2                 # contraction subtiles (F = 2 * 128)
NCORES = 8
NWARM = 8
CW = 512               # node-chunk width
NCHUNK = N // CW
WCOLS = KO * F + KO * N    # 4608 packed columns per partition

# packed-column helpers
def mcol(ko, dh):
    return ko * F + dh * P

def ccol(c, ko):
    return KO * F + c * KO * CW + ko * CW

# loads: (start_col, end_col). PE chunk c waits load LD_OF[c].
LOADS = [(0, ccol(1, 0)), (ccol(1, 0), ccol(2, 0)), (ccol(2, 0), WCOLS)]
LD_OF = [0, 1, 2, 2]

_NC = None
LAST_EXEC_TIME_NS = None
LAST_TRACE_PATH = None


def _ensure_axon_ntff_hook():
    """Make run_bass_kernel_spmd(trace=True) work under axon in this image
    (antenv.axon_hooks is absent; trn_boot carries the ctypes impl)."""
    try:
        import antenv.axon_hooks  # noqa: F401
        return
    except ImportError:
        pass
    try:
        from trn_agent_boot.trn_boot import _ntff_profile_via_ctypes

        hook = _ntff_profile_via_ctypes("/opt/axon/libaxon_pjrt.so")
        mod = types.ModuleType("antenv.axon_hooks")
        mod.get_axon_ntff_profile_hook = lambda: hook
        mod.set_axon_ntff_profile_hook = lambda h: None
        sys.modules["antenv.axon_hooks"] = mod
        import concourse.bass_utils as bass_utils

        bass_utils.upload_artifacts = lambda tmpdir: tmpdir  # no S3 here
    except Exception:
        pass


def _build_nc():
    from concourse import bacc, mybir

    bf16 = mybir.dt.bfloat16
    f32 = mybir.dt.float32

    nc = bacc.Bacc()
    hm = nc.declare_dram_parameter("hm", [P, WCOLS], bf16, isOutput=False)
    outT = nc.declare_dram_parameter("outT", [F, N], bf16, isOutput=True)

    oT_r = outT.rearrange("(dh p) n -> p dh n", p=P)   # (128, 2, 2048)

    es = ExitStack()
    h_sb = es.enter_context(nc.sbuf_tensor("h_sb", [P, WCOLS], bf16))
    o_sb = es.enter_context(nc.sbuf_tensor("o_sb", [P, KO, N], bf16))
    wu_sb = es.enter_context(nc.sbuf_tensor("wu_sb", [P, 512], bf16))
    psum = [
        es.enter_context(nc.psum_tensor(f"psum{i}", [P, CW], f32))
        for i in range(8)
    ]
    ld_sems = [
        es.enter_context(nc.semaphore(f"ld_sem{s}")) for s in range(len(LOADS))
    ]
    mm_sem = es.enter_context(nc.semaphore("mm_sem"))
    cpc = [es.enter_context(nc.semaphore(f"cpc{c}")) for c in range(NCHUNK)]
    st_sem = es.enter_context(nc.semaphore("st_sem"))  # DMA completion target
    # (nothing waits on st_sem: the NRT epilogue's drains + ~6.7us teardown
    #  run long after the ~1us store wire time)

    # ---- loads (sync ring, consumption order) ----
    for si, (a, b) in enumerate(LOADS):
        nc.sync.dma_start(h_sb[:, a:b], hm[:, a:b]).then_inc(ld_sems[si], 16)

    # ---- PE: HAM warmup on (garbage) wu_sb, then the 16 real matmuls ----
    for _ in range(NWARM):
        nc.tensor.matmul(psum[0][:], wu_sb[:, :P], wu_sb[:], start=True, stop=True)
    g = 0
    for c in range(NCHUNK):
        first = True
        for dh in range(KO):
            mm0 = nc.tensor.matmul(
                psum[g][:, :],
                h_sb[:, mcol(0, dh):mcol(0, dh) + P],
                h_sb[:, ccol(c, 0):ccol(c, 0) + CW],
                start=True,
                stop=False,
            )
            if first:
                mm0._wait_ge(ld_sems[LD_OF[c]], 16)
                first = False
            nc.tensor.matmul(
                psum[g][:, :],
                h_sb[:, mcol(1, dh):mcol(1, dh) + P],
                h_sb[:, ccol(c, 1):ccol(c, 1) + CW],
                start=False,
                stop=True,
            ).then_inc(mm_sem, 1)
            g += 1

    def copy(eng, g):
        c, dh = g // 2, g % 2
        dst = o_sb[:, dh, c * CW:(c + 1) * CW]
        if eng is nc.vector:
            inst = eng.tensor_copy(dst, psum[g][:, :])
        else:
            inst = eng.copy(dst, psum[g][:, :])
        inst._wait_ge(mm_sem, g + 1).then_inc(cpc[c], 1)

    def store(eng_ring, c):
        eng_ring.wait_ge(cpc[c], 2)
        eng_ring.dma_start(
            oT_r[:, :, c * CW:(c + 1) * CW], o_sb[:, :, c * CW:(c + 1) * CW]
        ).then_inc(st_sem, 16)

    # ---- DVE: even copies (dh0 of each chunk) ----
    for g in (0, 2, 4, 6):
        copy(nc.vector, g)
    # ---- ACT: odd copies interleaved with the c0/c1 stores ----
    copy(nc.scalar, 1)
    store(nc.scalar, 0)
    copy(nc.scalar, 3)
    store(nc.scalar, 1)
    copy(nc.scalar, 5)
    copy(nc.scalar, 7)
    # ---- sync ring: c2/c3 stores after its loads ----
    store(nc.sync, 2)
    store(nc.sync, 3)

    es.close()
    nc.finalize()
    return nc


def _pack_inputs(h, Mmat_bf):
    """Per-core hm[128, 4608]: [M_ko0|M_ko1| c0_ko0|c0_ko1| ... c3_ko1]."""
    m_part = Mmat_bf.reshape(KO, P, F).transpose(1, 0, 2).reshape(P, KO * F)
    maps = []
    for b in range(NCORES):
        ht = np.ascontiguousarray(h[b].T).astype(bfloat16)       # (256, 2048)
        cpart = (
            ht.reshape(KO, P, NCHUNK, CW)
            .transpose(1, 2, 0, 3)
            .reshape(P, KO * N)
        )
        maps.append({"hm": np.concatenate([m_part, cpart], axis=1)})
    return maps


def kernel(h, adj, W, alpha_res):
    global _NC, LAST_EXEC_TIME_NS, LAST_TRACE_PATH

    h = np.asarray(h, dtype=np.float32)
    W = np.asarray(W, dtype=np.float32)
    alpha = float(np.asarray(alpha_res))
    # adj is unused by the reference's math.

    # M = alpha * concat-heads(W) + (1 - alpha) * I  (residual folded in)
    Wc = W.transpose(1, 0, 2).reshape(F, F)
    Mmat_bf = (alpha * Wc + (1.0 - alpha) * np.eye(F, dtype=np.float32)).astype(
        bfloat16
    )

    trace = os.environ.get("BASS_TRACE", "").lower() in ("1", "true", "yes")
    if trace:
        _ensure_axon_ntff_hook()

    from concourse.bass_utils import run_bass_kernel_spmd

    if _NC is None:
        _NC = _build_nc()

    in_maps = _pack_inputs(h, Mmat_bf)
    res = run_bass_kernel_spmd(
        _NC, in_maps, core_ids=list(range(NCORES)), trace=trace
    )
    LAST_EXEC_TIME_NS = res.exec_time_ns
    if res.instructions_and_trace is not None:
        LAST_TRACE_PATH = res.instructions_and_trace[1]

    return np.ascontiguousarray(
        np.stack(
            [res.results[b]["outT"].T.astype(np.float32) for b in range(NCORES)]
        )
    )
